# revision 1
# baseline (speedup 1.0000x reference)
"""Multi-head self-attention (B=4, L=2048, D=512, H=4, Hd=128) on 8 TRN2 cores.

Sharding: core c handles batch b = c//2 and head-pair p = c%2 (heads 2p, 2p+1).
Each core computes a partial output y_part[b] = sum_{h in pair} ctx_h @ Wo_h.T;
host gathers: y[b] = y_part[core 2b] + y_part[core 2b+1] + bo.

Dataflow per core (all matmuls bf16 inputs, fp32 PSUM accumulation):
  xT [512,2048] (host-pretransposed)  ->  QT,KT [hd,L] and V [L,hd] via PE
  scoresT [k,L_q] = KT_blk.T @ QT     (k-major so softmax-normalizer needs no
  attnT = exp(scoresT/sqrt(hd))        transposes: rowsums via ones-matmul)
  ctxT [hd,L_q] += V_blk.T @ attnT    (accumulate over k blocks)
  r = ones.T @ attnT; ctxT *= (1/r)   (broadcast via DRAM-bounce DMA)
  y_blk [L_q,D] += ctxT_blk.T @ WoT_h (accumulate over the 2 heads)
"""
import numpy as np
import ml_dtypes

B, L, D = 4, 2048, 512
H, HD = 4, 128
NCORES = 8
QW = 512          # query window (matmul N / PSUM bank)
NQC = L // QW     # 4 query windows
NKB = L // 128    # 16 key blocks
NDC = D // 128    # 4 contraction chunks for projections
SCALE = 1.0 / np.sqrt(HD)

_COMPILED = None


def _build():
    import concourse.bass as bass
    import concourse.mybir as mybir
    import concourse.tile as tile
    from concourse import bacc

    F32 = mybir.dt.float32
    BF16 = mybir.dt.bfloat16
    AF = mybir.ActivationFunctionType

    nc = bacc.Bacc("TRN2", target_bir_lowering=False, debug=False,
                   num_devices=NCORES)
    xT_d = nc.dram_tensor("xT", [D, L], BF16, kind="ExternalInput")
    wqT_d = nc.dram_tensor("wqT", [D, 256], BF16, kind="ExternalInput")
    wkT_d = nc.dram_tensor("wkT", [D, 256], BF16, kind="ExternalInput")
    wvT_d = nc.dram_tensor("wvT", [D, 256], BF16, kind="ExternalInput")
    woT_d = nc.dram_tensor("woT", [256, D], BF16, kind="ExternalInput")
    bq_d = nc.dram_tensor("bq", [128, 2], F32, kind="ExternalInput")
    bk_d = nc.dram_tensor("bk", [128, 2], F32, kind="ExternalInput")
    bv_d = nc.dram_tensor("bv", [1, 256], F32, kind="ExternalInput")
    y_d = nc.dram_tensor("y", [L, D], F32, kind="ExternalOutput")

    with tile.TileContext(nc) as tc:
        with (
            tc.tile_pool(name="singles", bufs=1) as singles,
            tc.tile_pool(name="psp", bufs=2, space="PSUM") as psp,
            tc.tile_pool(name="pss", bufs=2, space="PSUM") as pss_pool,
            tc.tile_pool(name="psr", bufs=2, space="PSUM") as psr_pool,
            tc.tile_pool(name="attnp", bufs=12) as attnp,
            tc.tile_pool(name="recp", bufs=4) as recp,
            tc.tile_pool(name="rec128p", bufs=2) as rec128p,
            tc.tile_pool(name="yp", bufs=3) as yp,
            tc.tile_pool(name="drp", bufs=4, space="DRAM") as drp,
        ):
            # ---- load inputs ----
            xt_sb = singles.tile([128, NDC, L], BF16)
            for c in range(NDC):
                nc.sync.dma_start(xt_sb[:, c, :], xT_d[128 * c:128 * c + 128, :])
            wq_sb = singles.tile([128, NDC, 256], BF16)
            wk_sb = singles.tile([128, NDC, 256], BF16)
            wv_sb = singles.tile([128, NDC, 256], BF16)
            for c in range(NDC):
                nc.sync.dma_start(wq_sb[:, c, :], wqT_d[128 * c:128 * c + 128, :])
                nc.sync.dma_start(wk_sb[:, c, :], wkT_d[128 * c:128 * c + 128, :])
                nc.sync.dma_start(wv_sb[:, c, :], wvT_d[128 * c:128 * c + 128, :])
            wo_sb = singles.tile([128, 2, D], BF16)
            for h in range(2):
                nc.sync.dma_start(wo_sb[:, h, :], woT_d[128 * h:128 * h + 128, :])
            bq_sb = singles.tile([128, 2], F32)
            bk_sb = singles.tile([128, 2], F32)
            nc.sync.dma_start(bq_sb[:], bq_d[:])
            nc.sync.dma_start(bk_sb[:], bk_d[:])
            bv_sb = singles.tile([128, 256], F32)
            nc.sync.dma_start(
                bv_sb[:],
                bass.AP(tensor=bv_d.ap().tensor, offset=0, ap=[[0, 128], [1, 256]]))
            ones_sb = singles.tile([128, 1], BF16)
            nc.vector.memset(ones_sb[:], 1.0)

            # ---- projections ----
            qt_sb = singles.tile([128, 2, L], BF16)   # QT per head [hd, L]
            kt_sb = singles.tile([128, 2, L], BF16)
            v_sb = singles.tile([128, NKB, 256], BF16)  # V [k-part, kblk, 2*hd]

            for h in range(2):
                for (w_sb, b_sb, o_sb) in ((wq_sb, bq_sb, qt_sb),
                                           (wk_sb, bk_sb, kt_sb)):
                    for qc in range(NQC):
                        win = slice(QW * qc, QW * qc + QW)
                        ps = psp.tile([128, QW], F32, name=f"ps_p{h}{qc}",
                                      tag="psp")
                        for dc in range(NDC):
                            nc.tensor.matmul(
                                ps[:], w_sb[:, dc, 128 * h:128 * h + 128],
                                xt_sb[:, dc, win],
                                start=(dc == 0), stop=(dc == NDC - 1))
                        nc.vector.tensor_scalar_add(
                            o_sb[:, h, win], ps[:], b_sb[:, h:h + 1])
            for lb in range(NKB):
                ps = psp.tile([128, QW], F32, name=f"ps_v{lb}", tag="psp")
                for dc in range(NDC):
                    nc.tensor.matmul(
                        ps[:, 0:256], xt_sb[:, dc, 128 * lb:128 * lb + 128],
                        wv_sb[:, dc, :],
                        start=(dc == 0), stop=(dc == NDC - 1))
                nc.vector.tensor_add(v_sb[:, lb, :], ps[:, 0:256], bv_sb[:])

            # ---- attention ----
            ct_sb = singles.tile([128, 2, L], BF16)   # normalized ctxT [hd, L]
            for h in range(2):
                hs = slice(128 * h, 128 * h + 128)
                for qc in range(NQC):
                    win = slice(QW * qc, QW * qc + QW)
                    attn_tiles = []
                    for kk in range(NKB // 2):
                        ps_s = pss_pool.tile([128, 1024], F32,
                                             name=f"ps_s{h}{qc}{kk}", tag="pss")
                        k0 = 256 * kk
                        nc.tensor.matmul(ps_s[:, 0:512],
                                         kt_sb[:, h, k0:k0 + 128],
                                         qt_sb[:, h, win],
                                         start=True, stop=True)
                        nc.tensor.matmul(ps_s[:, 512:1024],
                                         kt_sb[:, h, k0 + 128:k0 + 256],
                                         qt_sb[:, h, win],
                                         start=True, stop=True)
                        at = attnp.tile([128, 1024], BF16,
                                        name=f"at{h}{qc}{kk}", tag="attn")
                        nc.scalar.activation(at[:], ps_s[:], AF.Exp, scale=SCALE)
                        attn_tiles.append(at)
                    ps_c = psp.tile([128, QW], F32, name=f"ps_c{h}{qc}",
                                    tag="psp")
                    for kk in range(NKB // 2):
                        at = attn_tiles[kk]
                        nc.tensor.matmul(ps_c[:], v_sb[:, 2 * kk, hs],
                                         at[:, 0:512],
                                         start=(kk == 0), stop=False)
                        nc.tensor.matmul(ps_c[:], v_sb[:, 2 * kk + 1, hs],
                                         at[:, 512:1024],
                                         start=False, stop=(kk == NKB // 2 - 1))
                    ps_r = psr_pool.tile([1, QW], F32, name=f"ps_r{h}{qc}",
                                         tag="psr")
                    for kk in range(NKB // 2):
                        at = attn_tiles[kk]
                        nc.tensor.matmul(ps_r[:], ones_sb[:], at[:, 0:512],
                                         start=(kk == 0), stop=False)
                        nc.tensor.matmul(ps_r[:], ones_sb[:], at[:, 512:1024],
                                         start=False, stop=(kk == NKB // 2 - 1))
                    rec = recp.tile([1, QW], F32, name=f"rec{h}{qc}", tag="rec")
                    nc.vector.reciprocal(rec[:], ps_r[:])
                    rdram = drp.tile([1, QW], F32, name=f"rd{h}{qc}", tag="rd")
                    nc.sync.dma_start(rdram[:], rec[:])
                    rd = rdram[:]
                    rb = bass.AP(tensor=rd.tensor, offset=rd.offset,
                                 ap=[[0, 128]] + [list(d) for d in rd.ap[1:]])
                    rec128 = rec128p.tile([128, QW], F32,
                                          name=f"rec128{h}{qc}", tag="rec128")
                    nc.sync.dma_start(rec128[:], rb)
                    nc.vector.tensor_mul(ct_sb[:, h, win], ps_c[:], rec128[:])

            # ---- output projection ----
            for qb in range(NKB):
                ps_y = psp.tile([128, D], F32, name=f"ps_y{qb}", tag="psp")
                nc.tensor.matmul(ps_y[:], ct_sb[:, 0, 128 * qb:128 * qb + 128],
                                 wo_sb[:, 0, :], start=True, stop=False)
                nc.tensor.matmul(ps_y[:], ct_sb[:, 1, 128 * qb:128 * qb + 128],
                                 wo_sb[:, 1, :], start=False, stop=True)
                ysb = yp.tile([128, D], F32, name=f"ysb{qb}", tag="ysb")
                nc.vector.tensor_copy(ysb[:], ps_y[:])
                nc.sync.dma_start(y_d[128 * qb:128 * qb + 128, :], ysb[:])

    nc.compile()
    return nc


def _get_compiled():
    global _COMPILED
    if _COMPILED is None:
        _COMPILED = _build()
    return _COMPILED


def kernel(x, Wq, bq, Wk, bk, Wv, bv, Wo, bo):
    from concourse.bass_utils import run_bass_kernel_spmd

    bf16 = ml_dtypes.bfloat16
    x = np.asarray(x, np.float32)
    Wq, Wk, Wv, Wo = (np.asarray(w, np.float32) for w in (Wq, Wk, Wv, Wo))
    bq, bk, bv, bo = (np.asarray(b, np.float32) for b in (bq, bk, bv, bo))

    xT = {b: np.ascontiguousarray(x[b].T).astype(bf16) for b in range(B)}
    WqT, WkT, WvT, WoT = (np.ascontiguousarray(W.T) for W in (Wq, Wk, Wv, Wo))

    in_maps = []
    for c in range(NCORES):
        b = c // 2
        p = c % 2
        hs = slice(256 * p, 256 * p + 256)
        in_maps.append({
            "xT": xT[b],
            "wqT": WqT[:, hs].astype(bf16),
            "wkT": WkT[:, hs].astype(bf16),
            "wvT": WvT[:, hs].astype(bf16),
            "woT": np.ascontiguousarray(WoT[hs, :]).astype(bf16),
            "bq": np.ascontiguousarray(bq[hs].reshape(2, 128).T),
            "bk": np.ascontiguousarray(bk[hs].reshape(2, 128).T),
            "bv": bv[hs].reshape(1, 256).copy(),
        })

    nc = _get_compiled()
    res = run_bass_kernel_spmd(nc, in_maps, list(range(NCORES)))
    y = np.empty((B, L, D), np.float32)
    for b in range(B):
        y[b] = res.results[2 * b]["y"] + res.results[2 * b + 1]["y"] + bo
    return y
